# revision 7
# baseline (speedup 1.0000x reference)
"""GPSA transformer block (ConViT-style) for TRN2, data-parallel over 8 cores.

Layout strategy: activations feature-major [C, T] (T = 8*196 tokens/core).
The five big GEMMs (qk, v, proj, fc1, fc2) run fp8-e4m3 DoubleRow (two
128-deep k-groups per instruction at 0.5 cycles/row = 4x bf16): weights are
pre-scaled x64 and quantized host-side into [K/256, 128, 2, M] pair layout;
activations are quantized to fp8 at the producing ACT op (LN out, gelu out,
attn-O out) in the same [128, k_pair, cols] layout. Dequant (1/64 or 1/1024)
folds into the PSUM-consuming op. The attention core (S = qk^T, PE transpose,
O = v^T St) stays bf16.

Attention per (h, g=batch-pair): S on PE -> exp on ACT -> 3D row-sum reduce +
reciprocal on DVE -> E*r on Pool -> PE-transpose -> (+sqb) on Pool/DVE ->
O = vT.T @ St on PE -> ACT copy (scale=(1-sig)*16) into fp8 O. The softmax
renorm attn/sum(attn) is exactly 1 and is skipped; sqb is pre-divided by
(1-sig) and the (1-sig) gate applied via the O-copy scale.

x1 (post-attention residual) stays SBUF-resident fp32: LN2 stats and the fc2
residual read it directly (no DRAM round-trip). LN1 is software-pipelined one
chunk ahead of the qk/v GEMMs so the PE never waits on the normalize chain.
"""
import sys

sys.path.insert(0, "/opt/trn_rl_repo")

import numpy as np
import ml_dtypes

import concourse.bass as bass
import concourse.mybir as mybir
from concourse import tile
from concourse.masks import make_identity

F32 = mybir.dt.float32
BF16 = mybir.dt.bfloat16
FP8 = mybir.dt.float8e4
AF = mybir.ActivationFunctionType
ALU = mybir.AluOpType
DR = mybir.MatmulPerfMode.DoubleRow

B, N, C, H = 64, 196, 768, 16
D = 48
DP = 64            # padded head dim
CP = H * DP        # 1024 padded attention channels
FF = 4 * C         # 3072
NCORES = 8
BLOC = B // NCORES  # 8 batches per core
T = BLOC * N        # 1568 tokens per core
TCH = 392           # token chunk = 2 batches (PSUM bank = 512 fp32)
NCHUNK = T // TCH   # 4
KC = C // 128       # 6 k-tiles over C
KCP = CP // 128     # 8 k-tiles over padded C'
KFF = FF // 128     # 24
PC = KC // 2        # 3 k-pairs over C
PCP = KCP // 2      # 4 k-pairs over C'
PFF = KFF // 2      # 12 k-pairs over FF
SCALE = float(D) ** -0.5
EPS = 1e-5
NT = ((0, 128), (128, 68))  # token split within one batch
WS = 64.0           # weight pre-scale for fp8
OS = 16.0           # attention-O pre-scale for fp8


MAXW = 1  # walrus in this container rejects multi-wait instructions


class PatchedTileContext(tile.TileContext):
    """walrus in this container rejects >MAXW sync waits on one instruction
    ("Too many sync wait commands"). Split excess waits onto nofuse NoOps
    emitted just before the instruction on the same engine, and emit the
    exit-drain waits one per instruction."""

    def _add_instruction(self, inst):
        si = getattr(inst, "sync_info", None)
        waits = list(si.on_wait) if (si is not None and si.on_wait) else []
        if len(waits) > MAXW:
            extra = waits[:-MAXW]
            keep = waits[-MAXW:]
            for i in range(0, len(extra), MAXW):
                nop = mybir.InstNoOp(
                    name=f"{inst.name}_xw{i}",
                    sync_info=mybir.SyncInfo(
                        on_wait=extra[i : i + MAXW], on_update=[]),
                    bass_nofuse=True,
                    engine=inst.engine,
                )
                super()._add_instruction(nop)
            inst.sync_info = mybir.SyncInfo(
                on_wait=keep, on_update=list(si.on_update or []))
        super()._add_instruction(inst)

    def _drain_and_barrier(self, tick_clock, wait_clock):
        nc = self.nc
        clock = list(tick_clock.global_clock)
        for proc, sem in sorted(self.sems.allocated().items()):
            tick = clock[proc] if proc < len(clock) else 0
            if tick <= 0:
                continue
            mult = 16 if sem.name.startswith("DMA") else 1
            nc.sync.wait_ge(sem, tick * mult)
        nc.sync.drain()
        nc.all_engine_barrier()
        popped = nc._tile_sem_poison_stack.pop()
        assert popped is self._sem_poison
        nc.clear_and_free_semaphores(list(self.sems.allocated().values()))
        nc.all_engine_barrier()


def _pairs(w):
    """[K, M] -> [K/256 * 128, 2, M] fp8 DoubleRow pair layout, x WS."""
    k, m = w.shape
    q = (w * WS).astype(ml_dtypes.float8_e4m3)
    return np.ascontiguousarray(
        q.reshape(k // 256, 2, 128, m).transpose(0, 2, 1, 3).reshape(k // 2, 2, m))


def host_prep(inputs):
    """Preprocess full-problem numpy inputs into per-core DRAM tensor maps."""
    f32 = np.float32
    bf16 = ml_dtypes.bfloat16
    x = np.asarray(inputs["x"], f32)              # [B, N, C]
    qk_w = np.asarray(inputs["qk_w"], f32)        # [2C, C]
    v_w = np.asarray(inputs["v_w"], f32)          # [C, C]
    proj_w = np.asarray(inputs["proj_w"], f32)    # [C, C]
    fc1_w = np.asarray(inputs["fc1_w"], f32)      # [FF, C]
    fc2_w = np.asarray(inputs["fc2_w"], f32)      # [C, FF]

    # padded q/k lhsT: [C, 2*CP]; q head h -> cols 64h..64h+48, k -> CP + same
    wqk = np.zeros((C, 2 * CP), f32)
    for h in range(H):
        wqk[:, DP * h : DP * h + D] = qk_w[D * h : D * h + D, :].T
        wqk[:, CP + DP * h : CP + DP * h + D] = qk_w[C + D * h : C + D * h + D, :].T
    # padded v rhs: [C, CP] (token-major v output)
    wv = np.zeros((C, CP), f32)
    for h in range(H):
        wv[:, DP * h : DP * h + D] = v_w[D * h : D * h + D, :].T
    # padded proj lhsT: [CP, C]
    wproj = np.zeros((CP, C), f32)
    for h in range(H):
        wproj[DP * h : DP * h + D, :] = proj_w[:, D * h : D * h + D].T

    sig = 1.0 / (1.0 + np.exp(-np.asarray(inputs["gating"], np.float64)))  # [H]
    one_m_sig = (1.0 - sig).astype(f32)

    # pos attention, batch-independent: sQbar[h,n,m] = sig_h*softmax_m(.)/(1-sig_h)
    s = int(N ** 0.5)
    ind = np.arange(s)[None, :] - np.arange(s)[:, None]
    indx = np.tile(ind, (s, s))
    indy = np.repeat(np.repeat(ind, s, axis=0), s, axis=1)
    rel = np.stack([indx, indy, indx ** 2 + indy ** 2], -1).astype(f32)  # [N,N,3]
    logits = rel @ np.asarray(inputs["pos_w"], f32).T + np.asarray(inputs["pos_b"], f32)
    logits = logits.transpose(2, 0, 1).astype(np.float64)  # [H, N, N]
    e = np.exp(logits - logits.max(-1, keepdims=True))
    posP = e / e.sum(-1, keepdims=True)
    sqb = (posP * (sig / np.maximum(1.0 - sig, 1e-20))[:, None, None]).astype(f32)
    sqb = np.ascontiguousarray(sqb.transpose(0, 2, 1))  # [h, m, n] (pre-transposed)

    common = {
        "wqk": _pairs(wqk),                            # [384, 2, 2CP]
        "wv": _pairs(wv),                              # [384, 2, CP]
        "wproj": _pairs(wproj),                        # [512, 2, C]
        "wfc1": _pairs(fc1_w.T.copy()),                # [384, 2, FF]
        "wfc2": _pairs(fc2_w.T.copy()),                # [1536, 2, C]
        "sqb": sqb.astype(bf16),                       # [H, N, N]
        "n1w": np.asarray(inputs["norm1_w"], f32).reshape(KC, 128).T.copy(),
        "n1b": np.asarray(inputs["norm1_b"], f32).reshape(KC, 128).T.copy(),
        "n2w": np.asarray(inputs["norm2_w"], f32).reshape(KC, 128).T.copy(),
        "n2b": np.asarray(inputs["norm2_b"], f32).reshape(KC, 128).T.copy(),
        # proj dequant is 1/(WS*OS); bias rides the PSUM via a bf16 matmul
        "projb": (np.asarray(inputs["proj_b"], f32) * WS * OS).reshape(1, C).astype(bf16),
        "fc1b": np.asarray(inputs["fc1_b"], f32).reshape(KFF, 128).T.copy(),
        "fc2b": (np.asarray(inputs["fc2_b"], f32) * WS).reshape(1, C).astype(bf16),
    }
    in_maps = []
    for c in range(NCORES):
        xs = x[c * BLOC : (c + 1) * BLOC].reshape(T, C).T.copy()  # [C, T]
        in_maps.append({"x": xs, **common})
    return in_maps, one_m_sig


def build_bass(one_m_sig):
    nc = bass.Bass()
    dram = {}
    for name, shape, dt in [
        ("x", [C, T], F32),
        ("wqk", [PC * 128, 2, 2 * CP], FP8),
        ("wv", [PC * 128, 2, CP], FP8),
        ("wproj", [PCP * 128, 2, C], FP8),
        ("wfc1", [PC * 128, 2, FF], FP8),
        ("wfc2", [PFF * 128, 2, C], FP8),
        ("sqb", [H, N, N], BF16),
        ("n1w", [128, KC], F32),
        ("n1b", [128, KC], F32),
        ("n2w", [128, KC], F32),
        ("n2b", [128, KC], F32),
        ("projb", [1, C], BF16),
        ("fc1b", [128, KFF], F32),
        ("fc2b", [1, C], BF16),
    ]:
        dram[name] = nc.declare_dram_parameter(name, shape, dt, isOutput=False)
    y_d = nc.declare_dram_parameter("y", [C, T], F32, isOutput=True)
    nc.stat1_d = nc.dram_tensor("stat1", [2, T], F32, kind="Internal")
    nc.stat2_d = nc.dram_tensor("stat2", [2, T], F32, kind="Internal")

    with PatchedTileContext(nc) as tc:
        build_body(nc, tc, dram, y_d, one_m_sig)
    return nc


def _ln_chunk(nc, temps, stats_ps, out_q, x6, ch, ones_col, w_sb, b_sb, stat_d):
    """LN over C for one token chunk. x6: 6 SBUF APs [128, TCH] fp32.
    Writes fp8 normalized output into out_q[:, ct, ch*TCH...]."""
    cs = bass.ts(ch, TCH)
    s1 = stats_ps.tile([1, TCH], F32, tag="s1", name="s1")
    s2 = stats_ps.tile([1, TCH], F32, tag="s2", name="s2")
    for ct in range(KC):
        xs_ = x6[ct]
        x2t = temps.tile([128, TCH], BF16, tag="x2t", name="x2t")
        nc.scalar.activation(x2t[:], xs_, AF.Square)
        xb = temps.tile([128, TCH], BF16, tag="xb", name="xb")
        nc.gpsimd.tensor_copy(xb[:], xs_)
        nc.tensor.matmul(
            s1[:], ones_col[:], xb[:], start=(ct == 0), stop=(ct == KC - 1))
        nc.tensor.matmul(
            s2[:], ones_col[:], x2t[:], start=(ct == 0), stop=(ct == KC - 1))
    mu = temps.tile([1, TCH], F32, tag="mu", name="mu")
    nc.vector.tensor_scalar_mul(mu[:], s1[:], 1.0 / C)
    ex2 = temps.tile([1, TCH], F32, tag="ex2", name="ex2")
    nc.vector.tensor_scalar_mul(ex2[:], s2[:], 1.0 / C)
    mu2 = temps.tile([1, TCH], F32, tag="mu2", name="mu2")
    nc.vector.tensor_mul(mu2[:], mu[:], mu[:])
    var = temps.tile([1, TCH], F32, tag="var", name="var")
    nc.vector.tensor_sub(var[:], ex2[:], mu2[:])
    std = temps.tile([1, TCH], F32, tag="std", name="std")
    nc.scalar.activation(std[:], var[:], AF.Sqrt, bias=nc.consts_eps[:])
    rstd = temps.tile([1, TCH], F32, tag="rstd", name="rstd")
    nc.vector.reciprocal(rstd[:], std[:])
    nc.sync.dma_start(out=stat_d[0:1, cs], in_=mu[:])
    nc.sync.dma_start(out=stat_d[1:2, cs], in_=rstd[:])
    mu_b = temps.tile([128, TCH], F32, tag="mu_b", name="mu_b")
    nc.gpsimd.dma_start(out=mu_b[:], in_=stat_d[0:1, cs].to_broadcast((128, TCH)))
    rstd_b = temps.tile([128, TCH], F32, tag="rstd_b", name="rstd_b")
    nc.gpsimd.dma_start(out=rstd_b[:], in_=stat_d[1:2, cs].to_broadcast((128, TCH)))
    for ct in range(KC):
        t1 = temps.tile([128, TCH], F32, tag="t1", name="t1")
        nc.vector.tensor_sub(t1[:], x6[ct], mu_b[:])
        t2 = temps.tile([128, TCH], F32, tag="t2", name="t2")
        nc.vector.scalar_tensor_tensor(
            t2[:], t1[:], w_sb[:, ct : ct + 1], rstd_b[:],
            op0=ALU.mult, op1=ALU.mult)
        nc.scalar.activation(
            out_q[:, ct, cs], t2[:], AF.Identity, bias=b_sb[:, ct : ct + 1])


def build_body(nc, tc, dram, y_d, one_m_sig):
    consts = tc.alloc_tile_pool(name="consts", bufs=1)
    temps = tc.alloc_tile_pool(name="temps", bufs=2)

    ident = consts.tile([128, 128], BF16, tag="ident", name="ident")
    make_identity(nc, ident[:])
    ones_col = consts.tile([128, 1], BF16, tag="ones_col", name="ones_col")
    nc.vector.memset(ones_col[:], 1.0)
    ones_row = consts.tile([1, TCH], BF16, tag="ones_row", name="ones_row")
    nc.vector.memset(ones_row[:], 1.0)
    eps_t = consts.tile([1, 1], F32, tag="eps_t", name="eps_t")
    nc.vector.memset(eps_t[:], EPS)
    nc.consts_eps = eps_t
    small = {}
    for nm in ("n1w", "n1b", "n2w", "n2b", "projb", "fc1b", "fc2b"):
        dt_ = BF16 if nm in ("projb", "fc2b") else F32
        t = consts.tile(list(dram[nm].shape), dt_, tag=nm, name=nm)
        nc.sync.dma_start(out=t[:], in_=dram[nm][:])
        small[nm] = t

    # ---- weights for qk/v (fp8 pair layout) ----
    pool_wqk = tc.alloc_tile_pool(name="wqk", bufs=1)
    wqk_sb = []
    for p in range(PC):
        t = pool_wqk.tile([128, 2, 2 * CP], FP8, tag=f"wqk{p}", name=f"wqk{p}")
        nc.sync.dma_start(out=t[:], in_=dram["wqk"][bass.ts(p, 128), :, :])
        wqk_sb.append(t)
    pool_wv = tc.alloc_tile_pool(name="wv", bufs=1)
    wv_sb = []
    for p in range(PC):
        t = pool_wv.tile([128, 2, CP], FP8, tag=f"wv{p}", name=f"wv{p}")
        nc.sync.dma_start(out=t[:], in_=dram["wv"][bass.ts(p, 128), :, :])
        wv_sb.append(t)

    # ---- Phase A: LN1 (staggered one chunk ahead) + qk/v GEMMs per chunk ----
    pool_xc = tc.alloc_tile_pool(name="xc", bufs=2)
    pool_xn1 = tc.alloc_tile_pool(name="xn1", bufs=1)
    xn1_q = pool_xn1.tile([128, KC, T], FP8, tag="xn1q", name="xn1q")
    stats_ps = tc.alloc_tile_pool(name="stats_ps", bufs=1, space="PSUM")
    pool_qkv = tc.alloc_tile_pool(name="qkv", bufs=1, side="right")
    qk_sb = [pool_qkv.tile([128, T], BF16, tag=f"qk{m}", name=f"qk{m}")
             for m in range(2 * KCP)]
    vT = [pool_qkv.tile([nn, CP], BF16, tag=f"vT{2 * b + i}", name=f"vT{2 * b + i}")
          for b in range(BLOC) for i, (no, nn) in enumerate(NT)]
    ps_qk = tc.alloc_tile_pool(name="ps_qk", bufs=3, space="PSUM")
    ps_v = tc.alloc_tile_pool(name="ps_v", bufs=3, space="PSUM")

    def load_x(ch):
        x6 = []
        for ct in range(KC):
            t = pool_xc.tile([128, TCH], F32, tag=f"xc{ct}", name=f"xc{ct}")
            nc.sync.dma_start(
                out=t[:], in_=dram["x"][bass.ts(ct, 128), bass.ts(ch, TCH)])
            x6.append(t[:])
        return x6

    xcur = load_x(0)
    xnxt = load_x(1)
    _ln_chunk(nc, temps, stats_ps, xn1_q, xcur, 0, ones_col,
              small["n1w"], small["n1b"], nc.stat1_d)
    for ch in range(NCHUNK):
        if ch + 2 < NCHUNK:
            xnew = load_x(ch + 2)
        else:
            xnew = None
        if ch + 1 < NCHUNK:
            _ln_chunk(nc, temps, stats_ps, xn1_q, xnxt, ch + 1, ones_col,
                      small["n1w"], small["n1b"], nc.stat1_d)
            xnxt = xnew
        cs = bass.ts(ch, TCH)
        # qk GEMM: 16 m-tiles of 128 channels, fp8 DoubleRow over 3 k-pairs
        for m in range(2 * KCP):
            ps = ps_qk.tile([128, TCH], F32, tag="psqk", name="psqk")
            for p in range(PC):
                nc.tensor.matmul(
                    ps[:], wqk_sb[p][:, :, bass.ts(m, 128)],
                    xn1_q[:, 2 * p : 2 * p + 2, cs],
                    start=(p == 0), stop=(p == PC - 1), perf_mode=DR)
            nc.vector.tensor_scalar_mul(qk_sb[m][:, cs], ps[:], 1.0 / WS)
        # v GEMM (token-major, per batch in this chunk)
        for b in (2 * ch, 2 * ch + 1):
            for half, (no, nn) in enumerate(NT):
                tok = N * b + no
                for nch in range(2):
                    ps = ps_v.tile([128, 512], F32, tag="psv", name="psv")
                    for p in range(PC):
                        nc.tensor.matmul(
                            ps[:nn], xn1_q[:, 2 * p : 2 * p + 2, tok : tok + nn],
                            wv_sb[p][:, :, bass.ts(nch, 512)],
                            start=(p == 0), stop=(p == PC - 1), perf_mode=DR)
                    nc.scalar.activation(
                        vT[2 * b + half][:nn, bass.ts(nch, 512)], ps[:nn],
                        AF.Copy, scale=1.0 / WS)
    ps_v.release()
    ps_qk.release()
    stats_ps.release()
    pool_xn1.release()
    pool_xc.release()
    pool_wv.release()
    pool_wqk.release()

    # proj weights + sqb
    pool_wproj = tc.alloc_tile_pool(name="wproj", bufs=1, side="right")
    wproj_sb = []
    for p in range(PCP):
        t = pool_wproj.tile([128, 2, C], FP8, tag=f"wproj{p}", name=f"wproj{p}")
        nc.sync.dma_start(out=t[:], in_=dram["wproj"][bass.ts(p, 128), :, :])
        wproj_sb.append(t)
    pool_x1 = tc.alloc_tile_pool(name="x1", bufs=1)
    x1_sb = [pool_x1.tile([128, T], F32, tag=f"x1_{i}", name=f"x1_{i}")
             for i in range(KC)]
    pool_sqb = tc.alloc_tile_pool(name="sqb", bufs=1)
    sqb_sb = []
    for h in range(H):
        t1 = pool_sqb.tile([128, N], BF16, tag=f"sqb{h}_0", name=f"sqb{h}_0")
        nc.sync.dma_start(out=t1[:], in_=dram["sqb"][h, 0:128, :])
        t2 = pool_sqb.tile([68, N], BF16, tag=f"sqb{h}_1", name=f"sqb{h}_1")
        nc.sync.dma_start(out=t2[:], in_=dram["sqb"][h, 128:196, :])
        sqb_sb.append((t1, t2))

    # ---- Phase B: attention + proj interleaved over batch pairs ----
    pool_O = tc.alloc_tile_pool(name="O", bufs=1, side="right")
    O_q = pool_O.tile([128, KCP, T], FP8, tag="Oq", name="Oq")
    attn_sm = tc.alloc_tile_pool(name="attn_sm", bufs=2)
    ps_sa = tc.alloc_tile_pool(name="ps_sa", bufs=2, space="PSUM")
    ps_sb = tc.alloc_tile_pool(name="ps_sb", bufs=1, space="PSUM")
    ps_t = tc.alloc_tile_pool(name="ps_t", bufs=2, space="PSUM")
    ps_o = tc.alloc_tile_pool(name="ps_o", bufs=1, space="PSUM")
    ps_p = tc.alloc_tile_pool(name="ps_p", bufs=2, space="PSUM")
    for g in range(NCHUNK):
        bb = 2 * g
        for h in range(H):
            qt = qk_sb[h // 2]
            kt = qk_sb[KCP + h // 2]
            ko = DP * (h % 2)
            sq1, sq2 = sqb_sb[h]
            oms = float(one_m_sig[h])
            psSa = ps_sa.tile([128, 2 * N], F32, tag="psSa", name="psSa")
            psSb = ps_sb.tile([68, 2 * N], F32, tag="psSb", name="psSb")
            for j in range(2):
                tb = N * (bb + j)
                nc.tensor.matmul(
                    psSa[:, N * j : N * j + N], qt[ko : ko + DP, tb : tb + 128],
                    kt[ko : ko + DP, tb : tb + N], start=True, stop=True)
                nc.tensor.matmul(
                    psSb[:, N * j : N * j + N], qt[ko : ko + DP, tb + 128 : tb + N],
                    kt[ko : ko + DP, tb : tb + N], start=True, stop=True)
            Ea = attn_sm.tile([128, 2 * N], BF16, tag="Ea", name="Ea")
            nc.scalar.activation(Ea[:], psSa[:], AF.Exp, scale=SCALE)
            Eb = attn_sm.tile([68, 2 * N], BF16, tag="Eb", name="Eb")
            nc.scalar.activation(Eb[:], psSb[:], AF.Exp, scale=SCALE)
            dra = attn_sm.tile([128, 2], F32, tag="dra", name="dra")
            drb = attn_sm.tile([68, 2], F32, tag="drb", name="drb")
            nc.vector.tensor_reduce(
                dra[:], Ea[:].rearrange("p (two n) -> p two n", two=2),
                axis=mybir.AxisListType.X, op=ALU.add)
            nc.vector.tensor_reduce(
                drb[:], Eb[:].rearrange("p (two n) -> p two n", two=2),
                axis=mybir.AxisListType.X, op=ALU.add)
            ra = attn_sm.tile([128, 2], F32, tag="ra", name="ra")
            nc.vector.reciprocal(ra[:], dra[:])
            rb = attn_sm.tile([68, 2], F32, tag="rb", name="rb")
            nc.vector.reciprocal(rb[:], drb[:])
            Sa = attn_sm.tile([128, 2 * N], BF16, tag="Sa", name="Sa")
            Sb = attn_sm.tile([68, 2 * N], BF16, tag="Sb", name="Sb")
            for j in range(2):
                js = bass.ds(N * j, N)
                nc.gpsimd.tensor_scalar(
                    Sa[:, js], Ea[:, js], ra[:, j : j + 1], None, op0=ALU.mult)
                nc.gpsimd.tensor_scalar(
                    Sb[:, js], Eb[:, js], rb[:, j : j + 1], None, op0=ALU.mult)
            psO = ps_o.tile([DP, 2 * N], F32, tag="psO", name="psO")
            for j in range(2):
                js = bass.ds(N * j, N)
                jo = N * j
                psT = ps_t.tile([128, 2 * N], BF16, tag="psT", name="psT")
                nc.tensor.transpose(
                    psT[0:128, 0:128], Sa[:, jo : jo + 128], ident[:])
                nc.tensor.transpose(
                    psT[0:128, 128:196], Sb[:, jo : jo + 128],
                    ident[0:68, 0:68])
                nc.tensor.transpose(
                    psT[0:68, N : N + 128], Sa[:, jo + 128 : jo + N], ident[:])
                nc.tensor.transpose(
                    psT[0:68, N + 128 : 2 * N], Sb[:, jo + 128 : jo + N],
                    ident[0:68, 0:68])
                Sts1 = attn_sm.tile([128, N], BF16, tag="Sts1", name="Sts1")
                nc.vector.tensor_tensor(Sts1[:], psT[:, 0:N], sq1[:], op=ALU.add)
                Sts2 = attn_sm.tile([68, N], BF16, tag="Sts2", name="Sts2")
                nc.vector.tensor_tensor(
                    Sts2[:], psT[0:68, N : 2 * N], sq2[:], op=ALU.add)
                tb2 = 2 * (bb + j)
                nc.tensor.matmul(psO[:, js], vT[tb2][:, DP * h : DP * h + DP],
                                 Sts1[:], start=True, stop=False)
                nc.tensor.matmul(psO[:, js],
                                 vT[tb2 + 1][:68, DP * h : DP * h + DP],
                                 Sts2[:], start=False, stop=True)
            nc.scalar.activation(
                O_q[ko : ko + DP, h // 2, N * bb : N * bb + 2 * N], psO[:],
                AF.Copy, scale=oms * OS)
        # ---- proj + residual for this token chunk (pipelines with attn) ----
        cs = bass.ts(g, TCH)
        for m in range(KC):
            ps = ps_p.tile([128, TCH], F32, tag="psP", name="psP")
            nc.tensor.matmul(
                ps[:], small["projb"][:, bass.ts(m, 128)],
                ones_row[:], start=True, stop=False)
            for p in range(PCP):
                nc.tensor.matmul(
                    ps[:], wproj_sb[p][:, :, bass.ts(m, 128)],
                    O_q[:, 2 * p : 2 * p + 2, cs],
                    start=False, stop=(p == PCP - 1), perf_mode=DR)
            xres = temps.tile([128, TCH], F32, tag="xres", name="xres")
            nc.sync.dma_start(out=xres[:], in_=dram["x"][bass.ts(m, 128), cs])
            nc.vector.scalar_tensor_tensor(
                x1_sb[m][:, cs], ps[:], 1.0 / (WS * OS), xres[:],
                op0=ALU.mult, op1=ALU.add)
    ps_p.release()
    ps_o.release()
    ps_t.release()
    ps_sb.release()
    ps_sa.release()
    attn_sm.release()
    pool_sqb.release()
    pool_O.release()
    pool_wproj.release()
    pool_qkv.release()

    # fc weights (fp8 pair layout)
    pool_wfc = tc.alloc_tile_pool(name="wfc", bufs=1, side="right")
    wfc1_sb = []
    for p in range(PC):
        t = pool_wfc.tile([128, 2, FF], FP8, tag=f"wfc1_{p}", name=f"wfc1_{p}")
        nc.sync.dma_start(out=t[:], in_=dram["wfc1"][bass.ts(p, 128), :, :])
        wfc1_sb.append(t)
    wfc2_sb = []
    for p in range(PFF):
        t = pool_wfc.tile([128, 2, C], FP8, tag=f"wfc2_{p}", name=f"wfc2_{p}")
        nc.sync.dma_start(out=t[:], in_=dram["wfc2"][bass.ts(p, 128), :, :])
        wfc2_sb.append(t)

    # ---- Phase C: LN2 (from SBUF x1) then MLP + residual -> y ----
    pool_xn2 = tc.alloc_tile_pool(name="xn2", bufs=1, side="right")
    xn2_q = pool_xn2.tile([128, KC, T], FP8, tag="xn2q", name="xn2q")
    stats_ps2 = tc.alloc_tile_pool(name="stats_ps2", bufs=1, space="PSUM")
    for ch in range(NCHUNK):
        cs = bass.ts(ch, TCH)
        _ln_chunk(nc, temps, stats_ps2, xn2_q, [x1_sb[i][:, cs] for i in range(KC)],
                  ch, ones_col, small["n2w"], small["n2b"], nc.stat2_d)
    stats_ps2.release()

    pool_hdn = tc.alloc_tile_pool(name="hdn", bufs=2)
    ps_f1 = tc.alloc_tile_pool(name="ps_f1", bufs=2, space="PSUM")
    ps_f2 = tc.alloc_tile_pool(name="ps_f2", bufs=2, space="PSUM")
    for ch in range(NCHUNK):
        cs = bass.ts(ch, TCH)
        hdn_q = pool_hdn.tile([128, KFF, TCH], FP8, tag="hdnq", name="hdnq")
        for m in range(KFF):
            ps = ps_f1.tile([128, TCH], F32, tag="psF1", name="psF1")
            for p in range(PC):
                nc.tensor.matmul(
                    ps[:], wfc1_sb[p][:, :, bass.ts(m, 128)],
                    xn2_q[:, 2 * p : 2 * p + 2, cs],
                    start=(p == 0), stop=(p == PC - 1), perf_mode=DR)
            nc.scalar.activation(hdn_q[:, m, :], ps[:], AF.Gelu,
                                 scale=1.0 / WS, bias=small["fc1b"][:, m : m + 1])
        for m in range(KC):
            ps = ps_f2.tile([128, TCH], F32, tag="psF2", name="psF2")
            nc.tensor.matmul(
                ps[:], small["fc2b"][:, bass.ts(m, 128)],
                ones_row[:], start=True, stop=False)
            for p in range(PFF):
                nc.tensor.matmul(
                    ps[:], wfc2_sb[p][:, :, bass.ts(m, 128)],
                    hdn_q[:, 2 * p : 2 * p + 2, :],
                    start=False, stop=(p == PFF - 1), perf_mode=DR)
            ych = temps.tile([128, TCH], F32, tag="ych", name="ych")
            nc.vector.scalar_tensor_tensor(
                ych[:], ps[:], 1.0 / WS, x1_sb[m][:, cs],
                op0=ALU.mult, op1=ALU.add)
            nc.sync.dma_start(out=y_d[bass.ts(m, 128), cs], in_=ych[:])
    ps_f2.release()
    ps_f1.release()
    pool_hdn.release()
    pool_xn2.release()
    pool_wfc.release()
    pool_x1.release()
    temps.release()
    consts.release()


def postprocess(results):
    """results: list of per-core out dicts with y [C, T] -> full [B, N, C]."""
    outs = []
    for c in range(NCORES):
        y = np.asarray(results[c]["y"])  # [C, T]
        outs.append(y.T.reshape(BLOC, N, C))
    return np.concatenate(outs, 0)


# ----------------------------------------------------------------------------
# Entry point: FULL inputs -> FULL output (8-core SPMD data-parallel).
# ----------------------------------------------------------------------------
_BUILD_CACHE = {}
LAST_RESULT = None


def kernel(**inputs) -> np.ndarray:
    global LAST_RESULT
    import os

    trace = os.environ.get("KERNEL_TRACE", "0") == "1"
    if trace:
        _install_ntff_shim()
    else:
        os.environ.setdefault("BASS_NEVER_TRACE", "1")
    from concourse.bass_utils import run_bass_kernel_spmd

    in_maps, oms = host_prep(inputs)
    key = tuple(np.asarray(oms, np.float64).tolist())
    nc = _BUILD_CACHE.get(key)
    if nc is None:
        nc = build_bass(oms)
        _BUILD_CACHE[key] = nc
    kw = {}
    if trace:
        kw = dict(trace=True, tmpdir=os.environ.get("KERNEL_TRACE_DIR", None))
    res = run_bass_kernel_spmd(nc, in_maps, list(range(NCORES)), **kw)
    LAST_RESULT = res
    return postprocess(res.results)


def _install_ntff_shim():
    """Register the NTFF profile hook that this image's antenv lacks."""
    import types

    import antenv
    from concourse import bass_utils

    bass_utils.upload_artifacts = lambda tmpdir: f"local:{tmpdir}"
    if "antenv.axon_hooks" in sys.modules:
        return
    mod = types.ModuleType("antenv.axon_hooks")
    mod._hook = None
    mod.set_axon_ntff_profile_hook = lambda hook: setattr(mod, "_hook", hook)
    mod.get_axon_ntff_profile_hook = lambda: mod._hook
    sys.modules["antenv.axon_hooks"] = mod
    antenv.axon_hooks = mod
    from trn_agent_boot.trn_boot import _ntff_profile_via_ctypes

    hook = _ntff_profile_via_ctypes("/opt/axon/libaxon_pjrt.so")
    if hook is not None:
        mod.set_axon_ntff_profile_hook(hook)
